# revision 7
# baseline (speedup 1.0000x reference)
"""DepGcn forward kernel for Trainium2 (Bass/Tile), 8-core data-parallel.

Math (per batch b, handled by one NeuronCore):
    t[i,e] = sum_j adj[i,j] * (hidden[j,e] + dep_embed[j,i,e])
    out[i,d] = t[i,:] @ W[:,d] + bias[d]

The reference materializes fusion = (hidden+dep) @ W ([N,N,D] sized); we
instead reduce over j first, which makes the kernel purely HBM-bound on
streaming dep_embed (33.5 MB/core).

Implementation notes:
  - dep_embed[b] is streamed with j on SBUF partitions:
    tile[j, (i_local, e)] <- dep[jc*128+j, i0:i0+32, :]  (2 MB per DMA,
    32 KB contiguous per partition).
  - The weighted j-reduction runs on the TensorEngine as diagonal-block
    matmuls: out4[m, (ii,e)] = sum_j adjT[j, i0+m] * dep[j, i0+ii, e]
    for groups of 4 i's (N=512 per matmul); only the diagonal m==ii
    strips are kept. PSUM accumulates the two 128-wide j-chunks.
  - Strip extraction PSUM->SBUF alternates VectorE / ScalarE.
  - term1 = adj @ hidden and the final projection (@W + bias) are small
    PE matmuls; bias is added via a K=1 matmul with a ones row.
"""

import numpy as np

B, N, D = 8, 256, 128
NCORES = 8
ICHUNK = 32   # i's per dep tile (2 MB DMAs)
IGROUP = 4    # i's per diagonal-block matmul (rhs N = IGROUP*D = 512)

_CACHE = {}


def _build_bass():
    import concourse.bass as bass
    import concourse.mybir as mybir
    import concourse.tile as tile
    from concourse import bacc
    from concourse.masks import make_identity

    f32 = mybir.dt.float32
    nc = bacc.Bacc("TRN2", target_bir_lowering=False, debug=False)

    hid_d = nc.dram_tensor("hidden", [N, D], f32, kind="ExternalInput").ap()
    adj_d = nc.dram_tensor("adj", [N, N], f32, kind="ExternalInput").ap()
    dep_d = nc.dram_tensor("dep", [N, N, D], f32, kind="ExternalInput").ap()
    w_d = nc.dram_tensor("weight", [D, D], f32, kind="ExternalInput").ap()
    b_d = nc.dram_tensor("bias", [1, D], f32, kind="ExternalInput").ap()
    out_d = nc.dram_tensor("out", [N, D], f32, kind="ExternalOutput").ap()

    with tile.TileContext(nc) as tc:
        with (
            tc.tile_pool(name="const", bufs=1) as cpool,
            tc.tile_pool(name="deps", bufs=4) as dpool,
            tc.tile_pool(name="accs", bufs=1) as apool,
            tc.tile_pool(name="psg", bufs=4, space="PSUM") as psg,
            tc.tile_pool(name="psm", bufs=2, space="PSUM") as psm,
        ):
            ident = cpool.tile([128, 128], f32, name="ident")
            make_identity(nc, ident[:])

            w_sb = cpool.tile([D, D], f32, name="w_sb")
            nc.sync.dma_start(w_sb[:], w_d[:])
            bias_sb = cpool.tile([1, D], f32, name="bias_sb")
            nc.sync.dma_start(bias_sb[:], b_d[:])
            ones_sb = cpool.tile([1, 128], f32, name="ones_sb")
            nc.gpsimd.memset(ones_sb[:], 1.0)

            # hidden[j,e] with j split into two 128-partition chunks
            hid_sb = cpool.tile([128, 2, D], f32, name="hid_sb")
            nc.sync.dma_start(hid_sb[:], hid_d.rearrange("(jc j) e -> j jc e", j=128))
            # adj[i,j] with i split into two halves on partitions
            adj_sb = cpool.tile([128, 2, N], f32, name="adj_sb")
            nc.sync.dma_start(adj_sb[:], adj_d.rearrange("(ih i) j -> i ih j", i=128))

            # adjT[jc][j, i] = adj[i, jc*128+j]  (PE transposes of 128x128 blocks)
            adjT = [cpool.tile([128, N], f32, name=f"adjT{jc}") for jc in range(2)]
            for jc in range(2):
                for ih in range(2):
                    ps = psm.tile([128, 128], f32, name="ps_tr", tag="psm")
                    nc.tensor.transpose(
                        ps[:], adj_sb[:, ih, jc * 128:(jc + 1) * 128], ident[:]
                    )
                    nc.vector.tensor_copy(adjT[jc][:, ih * 128:(ih + 1) * 128], ps[:])

            # term1[i,e] = sum_j adj[i,j] * hidden[j,e]
            t1_sb = cpool.tile([128, 2, D], f32, name="t1_sb")
            for ih in range(2):
                ps = psm.tile([128, D], f32, name="ps_t1", tag="psm")
                for jc in range(2):
                    nc.tensor.matmul(
                        ps[:],
                        adjT[jc][:, ih * 128:(ih + 1) * 128],
                        hid_sb[:, jc, :],
                        start=(jc == 0),
                        stop=(jc == 1),
                    )
                nc.vector.tensor_copy(t1_sb[:, ih, :], ps[:])

            # t2[i,e] = sum_j adj[i,j] * dep[j,i,e]
            # M=1 matmuls per i (out [1,128] at psum partition 0, packed 4 per
            # bank), whole-bank copies to a partition-0 staging row, then one
            # SBUF->SBUF DMA per chunk scatters staging into [128,.] layout.
            t2_sb = apool.tile([128, 2, D], f32, name="t2_sb")
            dep_r = dep_d.rearrange("(jc j) i e -> jc j (i e)", j=128)
            for c in range(N // ICHUNK):
                tiles = []
                for jc in range(2):
                    t = dpool.tile([128, ICHUNK * D], f32, name="dep_t")
                    nc.sync.dma_start(
                        t[:], dep_r[jc, :, c * ICHUNK * D:(c + 1) * ICHUNK * D]
                    )
                    tiles.append(t)
                stage = apool.tile([1, ICHUNK * D], f32, name="stage", bufs=2)
                for g in range(ICHUNK // IGROUP):
                    ps = psg.tile([1, IGROUP * D], f32, name="ps_g")
                    for m in range(IGROUP):
                        iloc = g * IGROUP + m
                        i = c * ICHUNK + iloc
                        for jc in range(2):
                            nc.tensor.matmul(
                                ps[0:1, m * D:(m + 1) * D],
                                adjT[jc][:, i:i + 1],
                                tiles[jc][:, iloc * D:(iloc + 1) * D],
                                start=(jc == 0),
                                stop=(jc == 1),
                            )
                    dst = stage[0:1, g * IGROUP * D:(g + 1) * IGROUP * D]
                    if g % 2 == 0:
                        nc.vector.tensor_copy(dst, ps[:])
                    else:
                        nc.scalar.copy(dst, ps[:])
                ih, il0 = divmod(c * ICHUNK, 128)
                nc.sync.dma_start(t2_sb[il0:il0 + ICHUNK, ih, :], stage[:])

            # out[i,:] = (t1 + t2) @ W + bias
            for ih in range(2):
                acc = apool.tile([128, D], f32, name=f"acc{ih}")
                nc.vector.tensor_add(acc[:], t2_sb[:, ih, :], t1_sb[:, ih, :])
                psT = psm.tile([128, 128], f32, name="ps_accT", tag="psm")
                nc.tensor.transpose(psT[:], acc[:], ident[:])
                accT = apool.tile([128, 128], f32, name=f"accT{ih}")
                nc.vector.tensor_copy(accT[:], psT[:])
                ps_out = psm.tile([128, D], f32, name="ps_out", tag="psm")
                nc.tensor.matmul(ps_out[:], accT[:], w_sb[:], start=True, stop=False)
                nc.tensor.matmul(
                    ps_out[:], ones_sb[:], bias_sb[:], start=False, stop=True
                )
                out_sb = apool.tile([128, D], f32, name=f"out{ih}")
                nc.vector.tensor_copy(out_sb[:], ps_out[:])
                nc.sync.dma_start(out_d[ih * 128:(ih + 1) * 128, :], out_sb[:])

    nc.compile()
    return nc


def _get_nc():
    if "nc" not in _CACHE:
        _CACHE["nc"] = _build_bass()
    return _CACHE["nc"]


def _get_runner():
    """Build (once) a sharded-jit callable running the bass NEFF on 8 cores.

    Mirrors concourse.bass2jax.run_bass_via_pjrt's multi-core branch, but
    exposes the jitted function + input ordering so callers can device_put
    inputs ahead of time and time pure device execution.
    """
    if "runner" in _CACHE:
        return _CACHE["runner"]

    import jax
    from jax.experimental.shard_map import shard_map
    from jax.sharding import Mesh, PartitionSpec

    import concourse.mybir as mybir
    from concourse import bass2jax

    nc = _get_nc()
    bass2jax.install_neuronx_cc_hook()

    partition_name = nc.partition_id_tensor.name if nc.partition_id_tensor else None
    in_names, out_names, out_avals, zero_outs = [], [], [], []
    for alloc in nc.m.functions[0].allocations:
        if not isinstance(alloc, mybir.MemoryLocationSet):
            continue
        name = alloc.memorylocations[0].name
        if alloc.kind == "ExternalInput":
            if name != partition_name:
                in_names.append(name)
        elif alloc.kind == "ExternalOutput":
            out_names.append(name)
            shape = tuple(alloc.tensor_shape)
            dtype = mybir.dt.np(alloc.dtype)
            out_avals.append(jax.core.ShapedArray(shape, dtype))
            zero_outs.append(np.zeros(shape, dtype))
    n_params = len(in_names)
    all_in_names = in_names + out_names
    if partition_name is not None:
        all_in_names = all_in_names + [partition_name]

    def _body(*args):
        operands = list(args)
        if partition_name is not None:
            operands.append(bass2jax.partition_id_tensor())
        outs = bass2jax._bass_exec_p.bind(
            *operands,
            out_avals=tuple(out_avals),
            in_names=tuple(all_in_names),
            out_names=tuple(out_names),
            lowering_input_output_aliases=(),
            sim_require_finite=True,
            sim_require_nnan=True,
            nc=nc,
        )
        return tuple(outs)

    devices = jax.devices()[:NCORES]
    mesh = Mesh(np.asarray(devices), ("core",))
    n_outs = len(out_names)
    sharded = jax.jit(
        shard_map(
            _body,
            mesh=mesh,
            in_specs=(PartitionSpec("core"),) * (n_params + n_outs),
            out_specs=(PartitionSpec("core"),) * n_outs,
            check_rep=False,
        ),
        keep_unused=True,
    )
    _CACHE["runner"] = (sharded, in_names, out_names, out_avals, zero_outs, mesh)
    return _CACHE["runner"]


def _concat_inputs(hidden, adj, dep_embed, weight, bias):
    """Per-core input dict -> concatenated global arrays in in_names order."""
    per_core = {
        "hidden": hidden,
        "adj": adj,
        "dep": dep_embed,
        "weight": np.broadcast_to(weight[None], (NCORES,) + weight.shape),
        "bias": np.broadcast_to(bias[None], (NCORES,) + bias.shape),
    }
    _, in_names, _, _, _, _ = _get_runner()
    return [
        np.ascontiguousarray(
            per_core[n].reshape(-1, *per_core[n].shape[2:])
        )
        for n in in_names
    ]


def run_spmd(hidden, adj, dep_embed, weight, bias_weight):
    """Run the kernel on all 8 cores; returns out [B,N,D]."""
    hidden = np.ascontiguousarray(np.asarray(hidden), dtype=np.float32)
    adj = np.ascontiguousarray(np.asarray(adj), dtype=np.float32)
    dep_embed = np.ascontiguousarray(np.asarray(dep_embed), dtype=np.float32)
    weight = np.ascontiguousarray(np.asarray(weight), dtype=np.float32)
    bias = np.ascontiguousarray(np.asarray(bias_weight), dtype=np.float32).reshape(
        1, D
    )

    sharded, in_names, out_names, out_avals, zero_outs, mesh = _get_runner()
    concat_in = _concat_inputs(hidden, adj, dep_embed, weight, bias)
    concat_zeros = [
        np.zeros((NCORES * z.shape[0], *z.shape[1:]), z.dtype) for z in zero_outs
    ]
    out_arrs = sharded(*concat_in, *concat_zeros)
    oi = out_names.index("out")
    out = np.asarray(out_arrs[oi]).reshape(NCORES, *out_avals[oi].shape)
    return out.astype(np.float32)


def kernel(hidden, adj, dep_embed, weight, bias_weight):
    return run_spmd(hidden, adj, dep_embed, weight, bias_weight)
